# revision 16
# baseline (speedup 1.0000x reference)
"""Multi-head attention with RoPE on 8 Trainium2 NeuronCores.

Problem: B=2, S=2048, E=1024, H=16 heads, head_dim=64.
    q/k/v = x @ w_{q,k,v}.T ; RoPE(q), RoPE(k)
    att = softmax(q k^T / 8) ; y = (att v) @ w_o.T

Sharding: core c handles batch b = c//4 and head-quarter hq = c%4 (4 heads).
Each core computes a partial y through its 256 columns of w_o; the host sums
the 4 partials per batch (replacing an on-chip all-reduce) and stacks the
two batches. No collectives.

Per-core dataflow (all matmuls in float32r, fp32 PSUM accumulation):
  x.T via PE transpose -> Q.T/K.T (dim-major, 2-head pair tiles) + RoPE
  (PE pair-swap permutation + DVE mul/mul/add against host cos/sin tables),
  V token-major with a ones column (V_aug) so the attention-weight row sums
  ride along in the AV matmul. scoresT = K.T_h^T-slices @ Q.T_h (contraction
  over head_dim=64), exp on ScalarE (scale=1/8 fused), AV.T accumulated over
  k-tiles, normalization by the broadcast reciprocal of the ones-row sums,
  then y_partial = Z.T-slices @ w_o.T-slices.
"""
import os

import numpy as np

B, S, E, H = 2, 2048, 1024, 16
HD = E // H            # 64
N_CORES = 8
HQ = 4                 # heads per core
ET = E // 128          # 8 E tiles
TT = S // 128          # 16 token tiles
QC = S // 512          # 4 query chunks
KT = S // 128          # 16 key tiles


def _rope_tables_ref(seq_len, head_dim):
    """Bit-identical clone of reference._rope_tables computed through jax, so
    the fp32 rounding of theta (which is chaotic at theta ~ 1e7 rad) matches
    the reference exactly on this platform."""
    import jax.numpy as jnp

    dtype = jnp.float32
    i = jnp.arange(head_dim // 2, dtype=dtype)
    angles = jnp.power(jnp.asarray(10000.0, dtype), 2.0 * i / head_dim)
    theta = jnp.arange(seq_len, dtype=dtype)[:, None] * angles[None, :]
    return np.asarray(jnp.cos(theta)), np.asarray(jnp.sin(theta))  # (S, 32) f32


def _host_tables():
    cos, sin = _rope_tables_ref(S, HD)                   # (S, 32) float32
    cosf = np.empty((128, S), dtype=np.float32)
    sinf = np.empty((128, S), dtype=np.float32)
    for r in range(64):
        i = r // 2
        cosf[r] = cosf[r + 64] = cos[:, i]
        s = sin[:, i]
        sinf[r] = sinf[r + 64] = -s if r % 2 == 0 else s
    return cosf, sinf


def _perm_pair_swap():
    p = np.zeros((128, 128), dtype=np.float32)
    for i in range(128):
        p[i, i ^ 1] = 1.0
    return p


def split_excess_sync(nc, max_cmds=1):
    """The pinned walrus build allows one sync-wait command per instruction.
    Hoist extras onto NoOps inserted just before, on the same engine."""
    import concourse.mybir as mybir

    nid = [0]

    def mk_nop(engine, waits):
        nid[0] += 1
        return mybir.InstNoOp(
            name=f"I-syncsplit-{nid[0]}",
            sync_info=mybir.SyncInfo(on_wait=waits, on_update=[]),
            bass_nofuse=True,
            engine=engine,
        )

    import bass_rust

    for f in nc.m.functions:
        for blk in f.blocks:
            insts = blk.instructions
            out = []
            changed = False
            for inst in insts:
                si = inst.sync_info
                if si is None or inst.engine is None or type(inst).__name__ == "InstCall":
                    out.append(inst)
                    continue
                waits = list(si.on_wait)
                nw = len(waits)
                if nw > max_cmds:
                    for i in range(0, nw - max_cmds, max_cmds):
                        out.append(mk_nop(inst.engine, waits[i : i + max_cmds]))
                    inst.sync_info = bass_rust.SyncInfo(
                        on_wait=waits[nw - max_cmds :], on_update=list(si.on_update)
                    )
                    changed = True
                out.append(inst)
            if changed:
                blk.instructions = out


def build_kernel():
    import concourse.bass as bass
    import concourse.mybir as mybir
    import concourse.tile as tile

    f32 = mybir.dt.float32
    f32r = mybir.dt.float32r
    AF = mybir.ActivationFunctionType

    nc = bass.Bass("TRN2", target_bir_lowering=False, debug=False, num_devices=N_CORES)

    xb_ap = nc.dram_tensor("xb", [S, E], f32, kind="ExternalInput").ap()
    wqt_ap = nc.dram_tensor("wqt", [E, 256], f32, kind="ExternalInput").ap()
    wkt_ap = nc.dram_tensor("wkt", [E, 256], f32, kind="ExternalInput").ap()
    wvt_ap = nc.dram_tensor("wvt", [E, 256], f32, kind="ExternalInput").ap()
    wot_ap = nc.dram_tensor("wot", [256, E], f32, kind="ExternalInput").ap()
    cos_ap = nc.dram_tensor("cosf", [128, S], f32, kind="ExternalInput").ap()
    sin_ap = nc.dram_tensor("sinf", [128, S], f32, kind="ExternalInput").ap()
    id_ap = nc.dram_tensor("ident", [128, 128], f32, kind="ExternalInput").ap()
    pm_ap = nc.dram_tensor("perm", [128, 128], f32, kind="ExternalInput").ap()
    yp_ap = nc.dram_tensor("yp", [S, E], f32, kind="ExternalOutput").ap()

    with tile.TileContext(nc) as tc:
        with (
            tc.tile_pool(name="persist", bufs=1) as pp,
            tc.tile_pool(name="acc", bufs=4, space="PSUM") as pacc,
            tc.tile_pool(name="strm", bufs=4, space="PSUM") as pstrm,
        ):
            # ---- persistent tiles (live across both phases) ----
            ones_col = pp.tile([128, 1], f32)
            nc.vector.memset(ones_col[:], 1.0)
            ones1_r = pp.tile([1, 64], f32r)
            wot_r = pp.tile([128, 2 * E], f32r)
            vaug = pp.tile([128, HQ * KT * 65], f32r)
            qt = {}
            kt = {}
            for hp in range(2):
                qt[hp] = pp.tile([128, S], f32r, name=f"qt{hp}")
                kt[hp] = pp.tile([128, S], f32r, name=f"kt{hp}")
            zt = {}
            for zp in range(2):
                zt[zp] = pp.tile([128, S], f32r, name=f"zt{zp}")

            # ================= phase 1: x.T, projections, RoPE =============
            with (
                tc.tile_pool(name="tables", bufs=1) as pt,
                tc.tile_pool(name="wtmp", bufs=2) as pwt,
                tc.tile_pool(name="xsb", bufs=4) as pxs,
                tc.tile_pool(name="xtc", bufs=2) as pxt,
                tc.tile_pool(name="rope", bufs=2) as prp,
            ):
                ident = pt.tile([128, 128], f32)
                nc.sync.dma_start(ident[:], id_ap[:])
                perm_r = pt.tile([128, 128], f32r)
                perm_f = pwt.tile([128, 128], f32, tag="wtmp")
                nc.sync.dma_start(perm_f[:], pm_ap[:])
                nc.vector.tensor_copy(perm_r[:], perm_f[:])
                cosf = pt.tile([128, S], f32)
                sinf = pt.tile([128, S], f32)
                nc.sync.dma_start(cosf[:], cos_ap[:])
                nc.sync.dma_start(sinf[:], sin_ap[:])
                ones1_f = pwt.tile([1, 64], f32, tag="ones1")
                nc.vector.memset(ones1_f[:], 1.0)
                nc.vector.tensor_copy(ones1_r[:], ones1_f[:])

                # weights: w_[qkv].T per-E-tile slices side by side (f32r)
                w_r = {}
                for name, ap in (("q", wqt_ap), ("k", wkt_ap), ("v", wvt_ap)):
                    wr = pt.tile([128, ET * 256], f32r, name=f"w{name}r")
                    for e in range(ET):
                        wf = pwt.tile([128, 256], f32, tag="wtmp")
                        nc.sync.dma_start(wf[:], ap[e * 128 : (e + 1) * 128, :])
                        nc.vector.tensor_copy(wr[:, e * 256 : (e + 1) * 256], wf[:])
                    w_r[name] = wr
                for z in range(2):
                    wf = pwt.tile([128, E], f32, tag="wotmp")
                    nc.sync.dma_start(wf[:], wot_ap[z * 128 : (z + 1) * 128, :])
                    nc.scalar.copy(wot_r[:, z * E : (z + 1) * E], wf[:])

                # per 512-token chunk: build x.T chunk then project
                for qc in range(QC):
                    xtc = pxt.tile([128, ET * 512], f32r, tag="xtc")
                    for tk in range(4):
                        tt = qc * 4 + tk
                        xsb = pxs.tile([128, E], f32, tag="xsb")
                        nc.gpsimd.dma_start(xsb[:], xb_ap[tt * 128 : (tt + 1) * 128, :])
                        for e in range(ET):
                            ps = pstrm.tile([128, 512], f32, tag="ps")
                            nc.tensor.transpose(
                                ps[:, 0:128], xsb[:, e * 128 : (e + 1) * 128], ident[:]
                            )
                            dst = xtc[:, e * 512 + tk * 128 : e * 512 + (tk + 1) * 128]
                            if (tt * ET + e) % 2 == 0:
                                nc.vector.tensor_copy(dst, ps[:, 0:128])
                            else:
                                nc.scalar.copy(dst, ps[:, 0:128])

                    csl = slice(qc * 512, (qc + 1) * 512)
                    # Q.T / K.T head-pair chunks + RoPE
                    for name in ("q", "k"):
                        dstmap = qt if name == "q" else kt
                        for hp in range(2):
                            ps = pstrm.tile([128, 512], f32, tag="ps")
                            for e in range(ET):
                                nc.tensor.matmul(
                                    ps[:],
                                    w_r[name][:, e * 256 + hp * 128 : e * 256 + (hp + 1) * 128],
                                    xtc[:, e * 512 : (e + 1) * 512],
                                    start=(e == 0),
                                    stop=(e == ET - 1),
                                )
                            t_r = prp.tile([128, 512], f32r, tag="t_r")
                            nc.vector.tensor_copy(t_r[:], ps[:])
                            psrot = pstrm.tile([128, 512], f32, tag="ps")
                            nc.tensor.matmul(psrot[:], perm_r[:], t_r[:], start=True, stop=True)
                            m1 = prp.tile([128, 512], f32, tag="m1")
                            nc.vector.tensor_mul(m1[:], t_r[:], cosf[:, csl])
                            m2 = prp.tile([128, 512], f32, tag="m2")
                            nc.vector.tensor_mul(m2[:], psrot[:], sinf[:, csl])
                            nc.vector.tensor_add(dstmap[hp][:, csl], m1[:], m2[:])

                    # V token-major with ones column
                    for tk in range(4):
                        tt = qc * 4 + tk
                        ps = pstrm.tile([128, 512], f32, tag="ps")
                        for e in range(ET):
                            nc.tensor.matmul(
                                ps[:, 0:256],
                                xtc[:, e * 512 + tk * 128 : e * 512 + (tk + 1) * 128],
                                w_r["v"][:, e * 256 : (e + 1) * 256],
                                start=(e == 0),
                                stop=(e == ET - 1),
                            )
                        for h in range(HQ):
                            base = (h * KT + tt) * 65
                            nc.vector.tensor_copy(
                                vaug[:, base : base + 64], ps[:, h * 64 : (h + 1) * 64]
                            )
                            nc.vector.tensor_copy(
                                vaug[:, base + 64 : base + 65], ones_col[:]
                            )

            # ================= phase 2: attention + normalize + y ==========
            with (
                tc.tile_pool(name="exp", bufs=6) as pex,
                tc.tile_pool(name="norm", bufs=2) as pnm,
                tc.tile_pool(name="sums", bufs=1) as psm,
                tc.tile_pool(name="yout", bufs=3) as pyo,
            ):
                sums_all = {}
                recr_all = {}
                for h in range(HQ):
                    sums_all[h] = psm.tile([1, S], f32, name=f"sums{h}")
                    recr_all[h] = psm.tile([1, S], f32r, name=f"recr{h}")
                # per head: kt-outer / qc-inner so consecutive PE matmuls share
                # their stationary operand (K-block for the 4 score matmuls,
                # V-block for the 4 AV matmuls) -> weight reload elided, and the
                # 4 independent qc chains hide each exp behind other matmuls.
                for h in range(HQ):
                    hp, hr = h // 2, (h % 2) * 64
                    psav = {}
                    for qc in range(QC):
                        psav[qc] = pacc.tile([65, 512], f32, tag="av", name=f"av{qc}")
                    # software-pipelined emission: the 4 scores of k-tile k are
                    # emitted (and thus prioritized) before the exp/AV group of
                    # k-tile k-1, so the PE runs [4x scores | 4x AV] groups that
                    # each share one stationary operand (weight reload elided)
                    # while the exps overlap the scores group on ScalarE.
                    prev = None
                    for k in range(KT):
                        pss = {}
                        for qc in range(QC):
                            pss[qc] = pstrm.tile([128, 512], f32, tag="ps", name=f"ps{qc}")
                            nc.tensor.matmul(
                                pss[qc][:],
                                kt[hp][hr : hr + 64, k * 128 : (k + 1) * 128],
                                qt[hp][hr : hr + 64, qc * 512 : (qc + 1) * 512],
                                start=True,
                                stop=True,
                            )
                        if prev is not None:
                            kp, pss_p = prev
                            vbase = (h * KT + kp) * 65
                            for qc in range(QC):
                                et = pex.tile([128, 512], f32r, tag="et")
                                nc.scalar.activation(et[:], pss_p[qc][:], AF.Exp, scale=0.125)
                                nc.tensor.matmul(
                                    psav[qc][:],
                                    vaug[:, vbase : vbase + 65],
                                    et[:],
                                    start=(kp == 0),
                                    stop=False,
                                )
                        prev = (k, pss)
                    kp, pss_p = prev
                    vbase = (h * KT + kp) * 65
                    for qc in range(QC):
                        et = pex.tile([128, 512], f32r, tag="et")
                        nc.scalar.activation(et[:], pss_p[qc][:], AF.Exp, scale=0.125)
                        nc.tensor.matmul(
                            psav[qc][:],
                            vaug[:, vbase : vbase + 65],
                            et[:],
                            start=False,
                            stop=True,
                        )
                    # stash unnormalized Z + denominators; run the (slow, DVE-
                    # only) reciprocal now so it overlaps the next head's
                    # attention without touching the PE stream
                    for qc in range(QC):
                        qsl = slice(qc * 512, (qc + 1) * 512)
                        nc.vector.tensor_copy(sums_all[h][:, qsl], psav[qc][64:65, :])
                        nc.vector.tensor_copy(
                            zt[hp][hr : hr + 64, qsl], psav[qc][0:64, :]
                        )
                    for qc in range(QC):
                        qsl = slice(qc * 512, (qc + 1) * 512)
                        with nc.allow_low_precision(reason="softmax denom reciprocal"):
                            nc.vector.reciprocal(recr_all[h][:, qsl], sums_all[h][:, qsl])

                # tail, per query chunk: normalize all heads for this chunk,
                # then immediately project + store those 4 token tiles so the
                # output projection and DMA overlap the remaining normalizes
                for qc in range(QC):
                    qsl = slice(qc * 512, (qc + 1) * 512)
                    for h in range(HQ):
                        hp, hr = h // 2, (h % 2) * 64
                        psb = pstrm.tile([128, 512], f32, tag="ps")
                        nc.tensor.matmul(
                            psb[0:64, :], ones1_r[:], recr_all[h][:, qsl],
                            start=True, stop=True,
                        )
                        rbs = pnm.tile([128, 512], f32, tag="rbs")
                        nc.scalar.copy(rbs[hr : hr + 64, :], psb[0:64, :])
                        zsl = zt[hp][hr : hr + 64, qsl]
                        nc.vector.tensor_mul(zsl, zsl, rbs[hr : hr + 64, :])
                    for tk in range(4):
                        tt = qc * 4 + tk
                        yo = pyo.tile([128, E], f32, tag="yo")
                        for od in range(2):
                            psy = pstrm.tile([128, 512], f32, tag="ps")
                            for z in range(2):
                                nc.tensor.matmul(
                                    psy[:],
                                    zt[z][:, tt * 128 : (tt + 1) * 128],
                                    wot_r[:, z * E + od * 512 : z * E + (od + 1) * 512],
                                    start=(z == 0),
                                    stop=(z == 1),
                                )
                            if od == 0:
                                nc.vector.tensor_copy(yo[:, 0:512], psy[:])
                            else:
                                nc.scalar.copy(yo[:, 512:1024], psy[:])
                        nc.sync.dma_start(yp_ap[tt * 128 : (tt + 1) * 128, :], yo[:])
    split_excess_sync(nc)
    return nc


_NC_CACHE = None


def _get_nc():
    global _NC_CACHE
    if _NC_CACHE is None:
        _NC_CACHE = build_kernel()
    return _NC_CACHE


def _in_maps(x, w_q, w_k, w_v, w_o):
    cosf, sinf = _host_tables()
    ident = np.eye(128, dtype=np.float32)
    perm = _perm_pair_swap()
    maps = []
    for c in range(N_CORES):
        b, hq = c // HQ, c % HQ
        rows = slice(hq * 256, (hq + 1) * 256)
        maps.append(
            {
                "xb": np.ascontiguousarray(x[b]),
                "wqt": np.ascontiguousarray(w_q[rows, :].T),
                "wkt": np.ascontiguousarray(w_k[rows, :].T),
                "wvt": np.ascontiguousarray(w_v[rows, :].T),
                "wot": np.ascontiguousarray(w_o[:, rows].T),
                "cosf": cosf,
                "sinf": sinf,
                "ident": ident,
                "perm": perm,
            }
        )
    return maps


def _run(inputs, trace=False):
    from concourse.bass_utils import run_bass_kernel_spmd

    nc = _get_nc()
    maps = _in_maps(
        inputs["x"], inputs["w_q"], inputs["w_k"], inputs["w_v"], inputs["w_o"]
    )
    res = run_bass_kernel_spmd(nc, maps, list(range(N_CORES)), trace=trace)
    y = np.empty((B, S, E), dtype=np.float32)
    for b in range(B):
        acc = np.zeros((S, E), dtype=np.float64)
        for hq in range(HQ):
            acc += res.results[b * HQ + hq]["yp"]
        y[b] = acc.astype(np.float32)
    return y, res


def kernel(**inputs):
    y, _ = _run(inputs, trace=False)
    return y


# revision 17
# speedup vs baseline: 1.3467x; 1.3467x over previous
"""Multi-head attention with RoPE on 8 Trainium2 NeuronCores.

Problem: B=2, S=2048, E=1024, H=16 heads, head_dim=64.
    q/k/v = x @ w_{q,k,v}.T ; RoPE(q), RoPE(k)
    att = softmax(q k^T / 8) ; y = (att v) @ w_o.T

Sharding: core c handles batch b = c//4 and head-quarter hq = c%4 (4 heads).
Each core computes a partial y through its 256 columns of w_o; the host sums
the 4 partials per batch (replacing an on-chip all-reduce) and stacks the
two batches. No collectives.

Per-core dataflow (all matmuls in float32r, fp32 PSUM accumulation):
  x.T via PE transpose -> Q.T/K.T (dim-major, 2-head pair tiles) + RoPE
  (PE pair-swap permutation + DVE mul/mul/add against host cos/sin tables),
  V token-major with a ones column (V_aug) so the attention-weight row sums
  ride along in the AV matmul. scoresT = K.T_h^T-slices @ Q.T_h (contraction
  over head_dim=64), exp on ScalarE (scale=1/8 fused), AV.T accumulated over
  k-tiles, normalization by the broadcast reciprocal of the ones-row sums,
  then y_partial = Z.T-slices @ w_o.T-slices.
"""
import os

import numpy as np

B, S, E, H = 2, 2048, 1024, 16
HD = E // H            # 64
N_CORES = 8
HQ = 4                 # heads per core
ET = E // 128          # 8 E tiles
TT = S // 128          # 16 token tiles
QC = S // 512          # 4 query chunks
KT = S // 128          # 16 key tiles


def _rope_tables_ref(seq_len, head_dim):
    """Bit-identical clone of reference._rope_tables computed through jax, so
    the fp32 rounding of theta (which is chaotic at theta ~ 1e7 rad) matches
    the reference exactly on this platform."""
    import jax.numpy as jnp

    dtype = jnp.float32
    i = jnp.arange(head_dim // 2, dtype=dtype)
    angles = jnp.power(jnp.asarray(10000.0, dtype), 2.0 * i / head_dim)
    theta = jnp.arange(seq_len, dtype=dtype)[:, None] * angles[None, :]
    return np.asarray(jnp.cos(theta)), np.asarray(jnp.sin(theta))  # (S, 32) f32


def _host_tables():
    cos, sin = _rope_tables_ref(S, HD)                   # (S, 32) float32
    cosf = np.empty((128, S), dtype=np.float32)
    sinf = np.empty((128, S), dtype=np.float32)
    for r in range(64):
        i = r // 2
        cosf[r] = cosf[r + 64] = cos[:, i]
        s = sin[:, i]
        sinf[r] = sinf[r + 64] = -s if r % 2 == 0 else s
    return cosf, sinf


def _perm_pair_swap():
    p = np.zeros((128, 128), dtype=np.float32)
    for i in range(128):
        p[i, i ^ 1] = 1.0
    return p


def split_excess_sync(nc, max_cmds=1):
    """The pinned walrus build allows one sync-wait command per instruction.
    Hoist extras onto NoOps inserted just before, on the same engine."""
    import concourse.mybir as mybir

    nid = [0]

    def mk_nop(engine, waits):
        nid[0] += 1
        return mybir.InstNoOp(
            name=f"I-syncsplit-{nid[0]}",
            sync_info=mybir.SyncInfo(on_wait=waits, on_update=[]),
            bass_nofuse=True,
            engine=engine,
        )

    import bass_rust

    for f in nc.m.functions:
        for blk in f.blocks:
            insts = blk.instructions
            out = []
            changed = False
            for inst in insts:
                si = inst.sync_info
                if si is None or inst.engine is None or type(inst).__name__ == "InstCall":
                    out.append(inst)
                    continue
                waits = list(si.on_wait)
                nw = len(waits)
                if nw > max_cmds:
                    for i in range(0, nw - max_cmds, max_cmds):
                        out.append(mk_nop(inst.engine, waits[i : i + max_cmds]))
                    inst.sync_info = bass_rust.SyncInfo(
                        on_wait=waits[nw - max_cmds :], on_update=list(si.on_update)
                    )
                    changed = True
                out.append(inst)
            if changed:
                blk.instructions = out


def build_kernel():
    import concourse.bass as bass
    import concourse.mybir as mybir
    import concourse.tile as tile

    f32 = mybir.dt.float32
    f32r = mybir.dt.float32r
    AF = mybir.ActivationFunctionType

    nc = bass.Bass("TRN2", target_bir_lowering=False, debug=False, num_devices=N_CORES)

    xb_ap = nc.dram_tensor("xb", [S, E], f32, kind="ExternalInput").ap()
    wqt_ap = nc.dram_tensor("wqt", [E, 256], f32, kind="ExternalInput").ap()
    wkt_ap = nc.dram_tensor("wkt", [E, 256], f32, kind="ExternalInput").ap()
    wvt_ap = nc.dram_tensor("wvt", [E, 256], f32, kind="ExternalInput").ap()
    wot_ap = nc.dram_tensor("wot", [256, E], f32, kind="ExternalInput").ap()
    cos_ap = nc.dram_tensor("cosf", [128, S], f32, kind="ExternalInput").ap()
    sin_ap = nc.dram_tensor("sinf", [128, S], f32, kind="ExternalInput").ap()
    id_ap = nc.dram_tensor("ident", [128, 128], f32, kind="ExternalInput").ap()
    pm_ap = nc.dram_tensor("perm", [128, 128], f32, kind="ExternalInput").ap()
    yp_ap = nc.dram_tensor("yp", [S, E], f32, kind="ExternalOutput").ap()

    with tile.TileContext(nc) as tc:
        with (
            tc.tile_pool(name="persist", bufs=1) as pp,
            tc.tile_pool(name="acc", bufs=4, space="PSUM") as pacc,
            tc.tile_pool(name="strm", bufs=4, space="PSUM") as pstrm,
        ):
            # ---- persistent tiles (live across both phases) ----
            ones_col = pp.tile([128, 1], f32)
            nc.vector.memset(ones_col[:], 1.0)
            ones1_r = pp.tile([1, 64], f32r)
            wot_r = pp.tile([128, 2 * E], f32r)
            vaug = pp.tile([128, HQ * KT * 65], f32r)
            qt = {}
            kt = {}
            for hp in range(2):
                qt[hp] = pp.tile([128, S], f32r, name=f"qt{hp}")
                kt[hp] = pp.tile([128, S], f32r, name=f"kt{hp}")
            zt = {}
            for zp in range(2):
                zt[zp] = pp.tile([128, S], f32r, name=f"zt{zp}")

            # ================= phase 1: x.T, projections, RoPE =============
            with (
                tc.tile_pool(name="tables", bufs=1) as pt,
                tc.tile_pool(name="wtmp", bufs=2) as pwt,
                tc.tile_pool(name="xsb", bufs=4) as pxs,
                tc.tile_pool(name="xtc", bufs=2) as pxt,
                tc.tile_pool(name="rope", bufs=2) as prp,
            ):
                ident = pt.tile([128, 128], f32)
                nc.sync.dma_start(ident[:], id_ap[:])
                perm_r = pt.tile([128, 128], f32r)
                perm_f = pwt.tile([128, 128], f32, tag="wtmp")
                nc.sync.dma_start(perm_f[:], pm_ap[:])
                nc.vector.tensor_copy(perm_r[:], perm_f[:])
                cosf = pt.tile([128, S], f32)
                sinf = pt.tile([128, S], f32)
                nc.sync.dma_start(cosf[:], cos_ap[:])
                nc.sync.dma_start(sinf[:], sin_ap[:])
                ones1_f = pwt.tile([1, 64], f32, tag="ones1")
                nc.vector.memset(ones1_f[:], 1.0)
                nc.vector.tensor_copy(ones1_r[:], ones1_f[:])

                # weights: w_[qkv].T per-E-tile slices side by side (f32r)
                w_r = {}
                for name, ap in (("q", wqt_ap), ("k", wkt_ap), ("v", wvt_ap)):
                    wr = pt.tile([128, ET * 256], f32r, name=f"w{name}r")
                    for e in range(ET):
                        wf = pwt.tile([128, 256], f32, tag="wtmp")
                        nc.sync.dma_start(wf[:], ap[e * 128 : (e + 1) * 128, :])
                        nc.vector.tensor_copy(wr[:, e * 256 : (e + 1) * 256], wf[:])
                    w_r[name] = wr
                for z in range(2):
                    wf = pwt.tile([128, E], f32, tag="wotmp")
                    nc.sync.dma_start(wf[:], wot_ap[z * 128 : (z + 1) * 128, :])
                    nc.scalar.copy(wot_r[:, z * E : (z + 1) * E], wf[:])

                # per 512-token chunk: build x.T chunk then project
                for qc in range(QC):
                    xtc = pxt.tile([128, ET * 512], f32r, tag="xtc")
                    for tk in range(4):
                        tt = qc * 4 + tk
                        xsb = pxs.tile([128, E], f32, tag="xsb")
                        nc.sync.dma_start(xsb[:], xb_ap[tt * 128 : (tt + 1) * 128, :])
                        for e in range(ET):
                            ps = pstrm.tile([128, 512], f32, tag="ps")
                            nc.tensor.transpose(
                                ps[:, 0:128], xsb[:, e * 128 : (e + 1) * 128], ident[:]
                            )
                            dst = xtc[:, e * 512 + tk * 128 : e * 512 + (tk + 1) * 128]
                            if (tt * ET + e) % 2 == 0:
                                nc.vector.tensor_copy(dst, ps[:, 0:128])
                            else:
                                nc.scalar.copy(dst, ps[:, 0:128])

                    csl = slice(qc * 512, (qc + 1) * 512)
                    # Q.T / K.T head-pair chunks + RoPE
                    for name in ("q", "k"):
                        dstmap = qt if name == "q" else kt
                        for hp in range(2):
                            ps = pstrm.tile([128, 512], f32, tag="ps")
                            for e in range(ET):
                                nc.tensor.matmul(
                                    ps[:],
                                    w_r[name][:, e * 256 + hp * 128 : e * 256 + (hp + 1) * 128],
                                    xtc[:, e * 512 : (e + 1) * 512],
                                    start=(e == 0),
                                    stop=(e == ET - 1),
                                )
                            t_r = prp.tile([128, 512], f32r, tag="t_r")
                            nc.vector.tensor_copy(t_r[:], ps[:])
                            psrot = pstrm.tile([128, 512], f32, tag="ps")
                            nc.tensor.matmul(psrot[:], perm_r[:], t_r[:], start=True, stop=True)
                            m1 = prp.tile([128, 512], f32, tag="m1")
                            nc.vector.tensor_mul(m1[:], t_r[:], cosf[:, csl])
                            m2 = prp.tile([128, 512], f32, tag="m2")
                            nc.vector.tensor_mul(m2[:], psrot[:], sinf[:, csl])
                            nc.vector.tensor_add(dstmap[hp][:, csl], m1[:], m2[:])

                    # V token-major with ones column
                    for tk in range(4):
                        tt = qc * 4 + tk
                        ps = pstrm.tile([128, 512], f32, tag="ps")
                        for e in range(ET):
                            nc.tensor.matmul(
                                ps[:, 0:256],
                                xtc[:, e * 512 + tk * 128 : e * 512 + (tk + 1) * 128],
                                w_r["v"][:, e * 256 : (e + 1) * 256],
                                start=(e == 0),
                                stop=(e == ET - 1),
                            )
                        for h in range(HQ):
                            base = (h * KT + tt) * 65
                            nc.vector.tensor_copy(
                                vaug[:, base : base + 64], ps[:, h * 64 : (h + 1) * 64]
                            )
                            nc.vector.tensor_copy(
                                vaug[:, base + 64 : base + 65], ones_col[:]
                            )

            # ================= phase 2: attention + normalize + y ==========
            with (
                tc.tile_pool(name="exp", bufs=6) as pex,
                tc.tile_pool(name="norm", bufs=2) as pnm,
                tc.tile_pool(name="sums", bufs=1) as psm,
                tc.tile_pool(name="yout", bufs=3) as pyo,
            ):
                sums_all = {}
                recr_all = {}
                for h in range(HQ):
                    sums_all[h] = psm.tile([1, S], f32, name=f"sums{h}")
                    recr_all[h] = psm.tile([1, S], f32r, name=f"recr{h}")
                # per head: kt-outer / qc-inner so consecutive PE matmuls share
                # their stationary operand (K-block for the 4 score matmuls,
                # V-block for the 4 AV matmuls) -> weight reload elided, and the
                # 4 independent qc chains hide each exp behind other matmuls.
                for h in range(HQ):
                    hp, hr = h // 2, (h % 2) * 64
                    psav = {}
                    for qc in range(QC):
                        psav[qc] = pacc.tile([65, 512], f32, tag="av", name=f"av{qc}")
                    # software-pipelined emission: the 4 scores of k-tile k are
                    # emitted (and thus prioritized) before the exp/AV group of
                    # k-tile k-1, so the PE runs [4x scores | 4x AV] groups that
                    # each share one stationary operand (weight reload elided)
                    # while the exps overlap the scores group on ScalarE.
                    prev = None
                    for k in range(KT):
                        pss = {}
                        for qc in range(QC):
                            pss[qc] = pstrm.tile([128, 512], f32, tag="ps", name=f"ps{qc}")
                            nc.tensor.matmul(
                                pss[qc][:],
                                kt[hp][hr : hr + 64, k * 128 : (k + 1) * 128],
                                qt[hp][hr : hr + 64, qc * 512 : (qc + 1) * 512],
                                start=True,
                                stop=True,
                            )
                        if prev is not None:
                            kp, pss_p = prev
                            vbase = (h * KT + kp) * 65
                            for qc in range(QC):
                                et = pex.tile([128, 512], f32r, tag="et")
                                nc.scalar.activation(et[:], pss_p[qc][:], AF.Exp, scale=0.125)
                                nc.tensor.matmul(
                                    psav[qc][:],
                                    vaug[:, vbase : vbase + 65],
                                    et[:],
                                    start=(kp == 0),
                                    stop=False,
                                )
                        prev = (k, pss)
                    kp, pss_p = prev
                    vbase = (h * KT + kp) * 65
                    for qc in range(QC):
                        et = pex.tile([128, 512], f32r, tag="et")
                        nc.scalar.activation(et[:], pss_p[qc][:], AF.Exp, scale=0.125)
                        nc.tensor.matmul(
                            psav[qc][:],
                            vaug[:, vbase : vbase + 65],
                            et[:],
                            start=False,
                            stop=True,
                        )
                    # stash unnormalized Z + denominators; run the (slow, DVE-
                    # only) reciprocal now so it overlaps the next head's
                    # attention without touching the PE stream
                    for qc in range(QC):
                        qsl = slice(qc * 512, (qc + 1) * 512)
                        nc.vector.tensor_copy(sums_all[h][:, qsl], psav[qc][64:65, :])
                        nc.vector.tensor_copy(
                            zt[hp][hr : hr + 64, qsl], psav[qc][0:64, :]
                        )
                    for qc in range(QC):
                        qsl = slice(qc * 512, (qc + 1) * 512)
                        with nc.allow_low_precision(reason="softmax denom reciprocal"):
                            nc.vector.reciprocal(recr_all[h][:, qsl], sums_all[h][:, qsl])

                # tail, per query chunk: normalize all heads for this chunk,
                # then immediately project + store those 4 token tiles so the
                # output projection and DMA overlap the remaining normalizes
                for qc in range(QC):
                    qsl = slice(qc * 512, (qc + 1) * 512)
                    for h in range(HQ):
                        hp, hr = h // 2, (h % 2) * 64
                        psb = pstrm.tile([128, 512], f32, tag="ps")
                        nc.tensor.matmul(
                            psb[0:64, :], ones1_r[:], recr_all[h][:, qsl],
                            start=True, stop=True,
                        )
                        rbs = pnm.tile([128, 512], f32, tag="rbs")
                        nc.scalar.copy(rbs[hr : hr + 64, :], psb[0:64, :])
                        zsl = zt[hp][hr : hr + 64, qsl]
                        nc.vector.tensor_mul(zsl, zsl, rbs[hr : hr + 64, :])
                    for tk in range(4):
                        tt = qc * 4 + tk
                        yo = pyo.tile([128, E], f32, tag="yo")
                        for od in range(2):
                            psy = pstrm.tile([128, 512], f32, tag="ps")
                            for z in range(2):
                                nc.tensor.matmul(
                                    psy[:],
                                    zt[z][:, tt * 128 : (tt + 1) * 128],
                                    wot_r[:, z * E + od * 512 : z * E + (od + 1) * 512],
                                    start=(z == 0),
                                    stop=(z == 1),
                                )
                            if od == 0:
                                nc.vector.tensor_copy(yo[:, 0:512], psy[:])
                            else:
                                nc.scalar.copy(yo[:, 512:1024], psy[:])
                        nc.sync.dma_start(yp_ap[tt * 128 : (tt + 1) * 128, :], yo[:])
    split_excess_sync(nc)
    return nc


_NC_CACHE = None


def _get_nc():
    global _NC_CACHE
    if _NC_CACHE is None:
        _NC_CACHE = build_kernel()
    return _NC_CACHE


def _in_maps(x, w_q, w_k, w_v, w_o):
    cosf, sinf = _host_tables()
    ident = np.eye(128, dtype=np.float32)
    perm = _perm_pair_swap()
    maps = []
    for c in range(N_CORES):
        b, hq = c // HQ, c % HQ
        rows = slice(hq * 256, (hq + 1) * 256)
        maps.append(
            {
                "xb": np.ascontiguousarray(x[b]),
                "wqt": np.ascontiguousarray(w_q[rows, :].T),
                "wkt": np.ascontiguousarray(w_k[rows, :].T),
                "wvt": np.ascontiguousarray(w_v[rows, :].T),
                "wot": np.ascontiguousarray(w_o[:, rows].T),
                "cosf": cosf,
                "sinf": sinf,
                "ident": ident,
                "perm": perm,
            }
        )
    return maps


def _run(inputs, trace=False):
    from concourse.bass_utils import run_bass_kernel_spmd

    nc = _get_nc()
    maps = _in_maps(
        inputs["x"], inputs["w_q"], inputs["w_k"], inputs["w_v"], inputs["w_o"]
    )
    res = run_bass_kernel_spmd(nc, maps, list(range(N_CORES)), trace=trace)
    y = np.empty((B, S, E), dtype=np.float32)
    for b in range(B):
        acc = np.zeros((S, E), dtype=np.float64)
        for hq in range(HQ):
            acc += res.results[b * HQ + hq]["yp"]
        y[b] = acc.astype(np.float32)
    return y, res


def kernel(**inputs):
    y, _ = _run(inputs, trace=False)
    return y
